# revision 19
# baseline (speedup 1.0000x reference)
"""Trainium2 Bass kernel for nn_LowRankDynamicConv.

Math (low-rank refactor of the reference's dense dynamic conv):
  combined = [phrase_slot[b] | eos]                       [N, 2C]
  h        = relu(combined @ W1 + b1)                     [N, 4C]
  proj     = (h @ W2 + b2) viewed as [N*C, R]             [4096, 32]
  y[b]     = x[b] @ proj[b]   with x[b] = context [T, N*C] -> [T, R]
  out_k[t] = sum_j y[t + j - pad_k] @ kparam_k[:, :, j]   [T, C]
  out      = relu(LN(concat(out_k) @ Wo + bo))            [T, C]

Sharding: the contraction dim of the dominant matmul (y = x @ proj) is
column-sharded: core ci owns c in [ci*32, (ci+1)*32) and computes a
partial y for ALL 16 samples from x[:, :, :, c-shard] (32 MiB/core) and
its W2 column block (4 MiB/core instead of the full replicated 32 MiB).
Two ReduceScatters (even b's, then odd b's, so the first hides under
the second half of the stream) sum the partials and hand each core the
final y for its own 2 samples; stages 4/5 then run data-parallel.

x is pre-transposed on the host to [b, p, ch, t] so tiles DMA straight
into the [nc%128, t] layout stage 3 needs -- no on-chip transposes of
the big tensor.  f32 tiles are bitcast to f32r (same bits) at matmul
call sites, so nothing is ever converted on-chip.
"""
import sys

sys.path.insert(0, "/opt/trn_rl_repo")

import numpy as np

import concourse.bass as bass
import concourse.mybir as mybir
import concourse.tile as tile
from concourse import bacc
from concourse.bass_utils import run_bass_kernel_spmd
from concourse.masks import make_identity

F32 = mybir.dt.float32
F32R = mybir.dt.float32r
BF16 = mybir.dt.bfloat16
RELU = mybir.ActivationFunctionType.Relu
SQRT = mybir.ActivationFunctionType.Sqrt
ADD = mybir.AluOpType.add

NCORES = 8
B, T, N, C, R = 16, 1024, 16, 256, 32
CL = C // NCORES           # 32 c-channels per core
NCL = N * CL               # 512 contraction rows per core
CH = NCL // 128            # 4 chunks of 128
PAD = 2                    # max conv pad (k=5)
YW = T + 2 * PAD           # padded y width
TQ = 4                     # stage-4/5 t chunking
TCHUNK = T // TQ           # 256
KJ = [(1, [0]), (3, [0, 1, 2]), (5, [0, 1, 2, 3, 4])]
NJ = 9
# (kernel-block, j, pad) in Msb index order
JLIST = [(kb, j, k // 2) for kb, (k, js) in enumerate(KJ) for j in js]
EVENS = list(range(0, B, 2))
ODDS = list(range(1, B, 2))
ORDER = EVENS + ODDS


def _broadcast_ap(ap, parts):
    a = ap
    return bass.AP(tensor=a.tensor, offset=a.offset, ap=[[0, parts]] + list(a.ap))


def _r(ap):
    return ap.bitcast(F32R)


def _build():
    nc = bacc.Bacc("TRN2", num_devices=NCORES)

    xs = nc.dram_tensor("xs", [B, 128, CH, T], F32, kind="ExternalInput")
    phrase = nc.dram_tensor("phrase", [B * N, C], F32, kind="ExternalInput")
    eos = nc.dram_tensor("eos", [C], F32, kind="ExternalInput")
    w1 = nc.dram_tensor("w1", [2 * C, 4 * C], F32, kind="ExternalInput")
    b1 = nc.dram_tensor("b1", [4 * C], F32, kind="ExternalInput")
    w2s = nc.dram_tensor("w2s", [4 * C, CL * R], F32, kind="ExternalInput")
    b2s = nc.dram_tensor("b2s", [CL * R], F32, kind="ExternalInput")
    kjoin = nc.dram_tensor("kjoin", [NJ, R, C], F32, kind="ExternalInput")
    wo = nc.dram_tensor("wo", [3 * C, C], F32, kind="ExternalInput")
    bo = nc.dram_tensor("bo", [C], F32, kind="ExternalInput")
    gamma = nc.dram_tensor("gamma", [C], F32, kind="ExternalInput")
    beta = nc.dram_tensor("beta", [C], F32, kind="ExternalInput")
    out = nc.dram_tensor("out", [2, T, C], F32, kind="ExternalOutput")

    with tile.TileContext(nc) as tc:
        with tc.tile_pool(name="keep", bufs=1) as keep, \
             tc.tile_pool(name="dram", bufs=1, space="DRAM") as dram:
            ident = keep.tile([128, 128], F32)
            make_identity(nc, ident)
            identr = keep.tile([R, R], F32R)
            nc.vector.tensor_copy(identr, ident[0:R, 0:R])

            # persistent small weights
            kjr = keep.tile([R, NJ, C], F32R)
            nc.sync.dma_start(kjr, _r(kjoin[:, :, :].rearrange("j r d -> r j d")))
            wor = keep.tile([128, 6, C], F32R)
            nc.sync.dma_start(wor, _r(wo[:, :].rearrange("(fc p) co -> p fc co", p=128)))
            gsb = keep.tile([128, C], F32)
            nc.sync.dma_start(gsb, _broadcast_ap(gamma[:], 128))
            bsb = keep.tile([128, C], F32)
            nc.sync.dma_start(bsb, _broadcast_ap(beta[:], 128))
            bosb = keep.tile([128, C], F32)
            nc.sync.dma_start(bosb, _broadcast_ap(bo[:], 128))

            # final y for own samples, zero-padded at both ends
            ysb = []
            for g in range(2):
                y = keep.tile([R, YW], BF16, name=f"ysb{g}")
                nc.vector.memset(y, 0.0)
                ysb.append(y)
            # folded conv+output weights M_j = kj_{k,j} @ Wo_k, bf16
            msb = keep.tile([R, NJ, C], BF16)

            # stage-3 lhsT: [nc_loc%128, b, ch, r] with nc_loc = n*CL + c_loc
            projr = keep.tile([128, B, CH, R], F32R)

            # DRAM scratch
            pscr = dram.tile([B * N, CL * R], F32R)
            ydram = [dram.tile([NCORES, R, T], BF16, name=f"yd{g}") for g in range(2)]
            rsout = [dram.tile([1, R, T], BF16, name=f"rs{g}") for g in range(2)]

            # streaming pools open before phase A so x DMAs don't alias
            # phase-A SBUF (aliasing would serialize the stream behind it)
            with tc.tile_pool(name="pX", bufs=2) as pX, \
                 tc.tile_pool(name="yp", bufs=2, space="PSUM") as yp:

                # ---- phase A: proj shard for all samples ---------------------
                with tc.tile_pool(name="pA", bufs=1) as pA, \
                     tc.tile_pool(name="psA", bufs=2, space="PSUM") as psA:
                    phsb = pA.tile([128, 2, C], F32)
                    nc.sync.dma_start(
                        phsb, phrase[:, :].rearrange("(bb p) c -> p bb c", p=128))
                    eossb = pA.tile([128, 2], F32)
                    nc.sync.dma_start(eossb, eos[:].rearrange("(o p) -> p o", p=128))
                    w1sb = pA.tile([128, 4, 4 * C], F32R)
                    nc.sync.dma_start(
                        w1sb, _r(w1[:, :].rearrange("(ko p) m -> p ko m", p=128)))
                    b1sb = pA.tile([128, 8], F32)
                    nc.sync.dma_start(b1sb, b1[:].rearrange("(mo p) -> p mo", p=128))
                    w2sb = pA.tile([128, 8, CL * R], F32R)
                    nc.sync.dma_start(
                        w2sb, _r(w2s[:, :].rearrange("(ko p) q -> p ko q", p=128)))

                    # combined^T [c2%128, ko, bn]
                    combT = pA.tile([128, 4, B * N], F32R)
                    for ko in range(2):
                        for bb in range(2):
                            pt = psA.tile([128, 128], F32, tag="ph")
                            nc.tensor.transpose(
                                pt, phsb[:, bb, ko * 128:(ko + 1) * 128], ident)
                            nc.vector.tensor_copy(
                                combT[:, ko, bb * 128:(bb + 1) * 128], pt)
                    for o in range(2):
                        nc.vector.tensor_copy(
                            combT[:, 2 + o, :],
                            eossb[:, o:o + 1].to_broadcast((128, B * N)))

                    # h^T [m%128, mo, bn] = relu(W1^T combined + b1)
                    hT = pA.tile([128, 8, B * N], F32R)
                    for mo in range(8):
                        ph = psA.tile([128, B * N], F32, tag="h")
                        for ko in range(4):
                            nc.tensor.matmul(
                                ph, w1sb[:, ko, mo * 128:(mo + 1) * 128],
                                combT[:, ko, :],
                                start=(ko == 0), stop=(ko == 3))
                        nc.scalar.activation(out=hT[:, mo, :], in_=ph, func=RELU,
                                             bias=b1sb[:, mo:mo + 1], scale=1.0)

                    # proj shard [bn%128, bb, cols], cols = (c_loc, r):
                    # bias-add lands psum in SBUF already bounce-ready
                    b2bc = pA.tile([128, CL * R], F32)
                    nc.sync.dma_start(b2bc, _broadcast_ap(b2s[:], 128))
                    projt = pA.tile([128, 2, CL * R], F32R)
                    for bb in range(2):
                        for sl in range(2):
                            pp = psA.tile([128, 512], F32, tag="pj")
                            for ko in range(8):
                                nc.tensor.matmul(
                                    pp, hT[:, ko, bb * 128:(bb + 1) * 128],
                                    w2sb[:, ko, sl * 512:(sl + 1) * 512],
                                    start=(ko == 0), stop=(ko == 7))
                            nc.vector.tensor_add(
                                projt[:, bb, sl * 512:(sl + 1) * 512], pp,
                                b2bc[:, sl * 512:(sl + 1) * 512])

                    # bounce through DRAM: pscr[(b n), (c_loc r)]
                    nc.scalar.dma_start(
                        pscr[:, :].rearrange("(bb p) q -> p bb q", p=128), projt)
                    # fold kjoin @ Wo: per (j, dc) transpose kjr block then
                    # accumulate M_j[r, co] over the two d-chunks
                    kjT = pA.tile([128, NJ, 2, R], F32R)
                    jj = 0
                    for kb, (k, js) in enumerate(KJ):
                        for ji, j in enumerate(js):
                            for dc in range(2):
                                pt = psA.tile([128, 128], F32, tag="ph")
                                nc.tensor.transpose(
                                    _r(pt[:, 0:R]),
                                    kjr[:, jj + ji, dc * 128:(dc + 1) * 128],
                                    identr)
                                nc.vector.tensor_copy(
                                    kjT[:, jj + ji, dc, :], pt[:, 0:R])
                        jj += len(js)
                    for jx in range(NJ):
                        kb = 0 if jx < 1 else (1 if jx < 4 else 2)
                        pm = psA.tile([128, C], F32, tag="pj")
                        for dc in range(2):
                            nc.tensor.matmul(
                                pm[0:R, :], kjT[:, jx, dc, :],
                                wor[:, kb * 2 + dc, :],
                                start=(dc == 0), stop=(dc == 1))
                        nc.vector.tensor_copy(msb[:, jx, :], pm[0:R, :])

                    prd = pscr[:, :].rearrange(
                        "(b ch n4) (cl r) -> n4 cl b ch r",
                        b=B, ch=CH, n4=4, cl=CL, r=R)
                    for n4 in range(4):
                        nc.scalar.dma_start(projr[n4 * 32:(n4 + 1) * 32], prd[n4])

                # ---- phase X: stream x, partial y, ReduceScatter -------------
                # staging pools reuse phase-A SBUF (closed above)
                pGcm = tc.tile_pool(name="pG", bufs=1)
                pG = pGcm.__enter__()
                ygrp = [pG.tile([R, NCORES, T], BF16, name=f"yg{g}")
                        for g in range(2)]
                ogrp = [pG.tile([128, T // 128, C], F32, name=f"og{g}")
                        for g in range(2)]

                def stream_group(g, group):
                    for pi in range(4):          # pairs of samples
                        si = g * 8 + pi * 2
                        xT = pX.tile([128, 2, CH, T], F32R, tag="xT")
                        nc.sync.dma_start(
                            xT, _r(xs[si:si + 2]
                                   .rearrange("s p ch t -> p s ch t")))
                        for s in range(2):
                            b = group[pi * 2 + s]
                            for th in range(2):
                                py = yp.tile([R, 512], F32, tag="y")
                                for ch in range(CH):
                                    nc.tensor.matmul(
                                        py, projr[:, b, ch, :],
                                        xT[:, s, ch, th * 512:(th + 1) * 512],
                                        start=(ch == 0), stop=(ch == CH - 1))
                                nc.vector.tensor_copy(
                                    ygrp[g][:, pi * 2 + s,
                                            th * 512:(th + 1) * 512], py)
                    nc.gpsimd.dma_start(
                        ydram[g][:].rearrange("b r t -> r b t"), ygrp[g])

                def emit_rs(g):
                    nc.gpsimd.collective_compute(
                        "ReduceScatter", ADD,
                        replica_groups=[list(range(NCORES))],
                        ins=[ydram[g][:].opt()],
                        outs=[rsout[g][:].opt()])

                def stage45(g, pS, op):
                    # final y for own sample of this group
                    nc.gpsimd.dma_start(ysb[g][:, PAD:PAD + T], rsout[g][0])
                    for tb in range(T // 128):
                        t0 = tb * 128
                        po = op.tile([128, C], F32, tag="o")
                        for jx, (kb, j, pad) in enumerate(JLIST):
                            ys = ysb[g][:, PAD + t0 + j - pad:
                                        PAD + t0 + j - pad + 128]
                            nc.tensor.matmul(po, ys, msb[:, jx, :],
                                             start=(jx == 0), stop=(jx == NJ - 1))
                        osb = ogrp[g][:, tb, :]
                        nc.vector.tensor_add(osb, po, bosb)
                        st = pS.tile([128, 6], F32, tag="st")
                        nc.vector.bn_stats(out=st, in_=osb)
                        mv = pS.tile([128, 2], F32, tag="mv")
                        nc.vector.bn_aggr(out=mv, in_=st)
                        rs = pS.tile([128, 1], F32, tag="rs")
                        eps = pS.tile([128, 1], F32, tag="eps")
                        nc.vector.memset(eps, 1e-5)
                        nc.scalar.activation(out=rs, in_=mv[:, 1:2], func=SQRT,
                                             bias=eps, scale=1.0)
                        nc.vector.reciprocal(rs, rs)
                        nc.vector.tensor_scalar(osb, osb, mv[:, 0:1], rs,
                                                mybir.AluOpType.subtract,
                                                mybir.AluOpType.mult)
                        nc.vector.tensor_mul(osb, osb, gsb)
                        nc.vector.tensor_add(osb, osb, bsb)
                        nc.vector.tensor_scalar_max(osb, osb, 0.0)
                    nc.sync.dma_start(
                        out[g].rearrange("(tb p) co -> p tb co", p=128), ogrp[g])

                stream_group(0, EVENS)
                emit_rs(0)
                stream_group(1, ODDS)
                emit_rs(1)
                with tc.tile_pool(name="pS", bufs=3) as pS, \
                     tc.tile_pool(name="op", bufs=4, space="PSUM") as op:
                    stage45(0, pS, op)
                    stage45(1, pS, op)
                pGcm.__exit__(None, None, None)

    nc.compile()
    return nc


_NC = None


def _get_nc():
    global _NC
    if _NC is None:
        _NC = _build()
    return _NC


def _shard(inputs):
    """Split full inputs into per-core input maps (layout-only numpy work)."""
    x = np.ascontiguousarray(inputs["context_emb"], dtype=np.float32)
    assert x.shape == (B, T, N, C)
    kjoin = np.ascontiguousarray(np.concatenate(
        [np.moveaxis(inputs[f"k{k}"], 2, 0) for k in (1, 3, 5)], axis=0),
        dtype=np.float32)  # [9, 32, 256]
    w2 = np.ascontiguousarray(inputs["W2"], dtype=np.float32)
    b2 = np.ascontiguousarray(inputs["b2"], dtype=np.float32)
    shared = {
        "phrase": np.ascontiguousarray(
            inputs["phrase_slot"].reshape(B * N, C), dtype=np.float32),
        "eos": np.ascontiguousarray(inputs["eos_slot"].reshape(C), dtype=np.float32),
        "w1": np.ascontiguousarray(inputs["W1"], dtype=np.float32),
        "b1": np.ascontiguousarray(inputs["b1"], dtype=np.float32),
        "kjoin": kjoin,
        "wo": np.ascontiguousarray(inputs["Wo"], dtype=np.float32),
        "bo": np.ascontiguousarray(inputs["bo"], dtype=np.float32),
        "gamma": np.ascontiguousarray(inputs["gamma"], dtype=np.float32),
        "beta": np.ascontiguousarray(inputs["beta"], dtype=np.float32),
    }
    in_maps = []
    for ci in range(NCORES):
        m = dict(shared)
        xc = x[:, :, :, ci * CL:(ci + 1) * CL]        # [B, T, N, CL]
        xt = xc.transpose(0, 2, 3, 1)                 # [B, N, CL, T]
        xt = xt.reshape(B, CH, 128, T)                # [B, ch, p, T]
        xt = xt.transpose(0, 2, 1, 3)                 # [B, p, ch, T]
        m["xs"] = np.ascontiguousarray(xt[ORDER], dtype=np.float32)
        m["w2s"] = np.ascontiguousarray(
            w2[:, ci * CL * R:(ci + 1) * CL * R], dtype=np.float32)
        m["b2s"] = np.ascontiguousarray(
            b2[ci * CL * R:(ci + 1) * CL * R], dtype=np.float32)
        in_maps.append(m)
    return in_maps


def _run(inputs, **kwargs):
    nc = _get_nc()
    res = run_bass_kernel_spmd(nc, _shard(inputs), core_ids=list(range(NCORES)),
                               **kwargs)
    outs = [r["out"] for r in res.results]
    full = np.concatenate(outs, axis=0).reshape(B, T, C)
    return full, res


def kernel(**inputs) -> np.ndarray:
    out, _ = _run(inputs)
    return out
